# revision 43
# baseline (speedup 1.0000x reference)
"""Causal self-attention on 8 TRN2 NeuronCores, batch-data-parallel (one batch
element per core).

Schedule (evolved by trace analysis from a 271us baseline to ~235us):
  - wqk host-interleaved per head pair ([q_j | k_j] 256-col blocks), few
    large priority-ordered DMAs (each DMA_DIRECT2D costs ~0.5us of Sync
    issue time): the first QK chain depends on ~0.75MB instead of 6MB, so
    the PE starts at ~9us instead of 20us; 16 junk warm-up matmuls during
    the DMA prologue lift the HAM clock gate (1.2->2.4GHz) early.
  - QKV fill chains rebalanced: pair j computes pair j+1's qk chains, v
    chains spread over pairs 0-3, pops spread to odd steps so pairs 4-6
    stay PE-dense and HAM never re-throttles mid-kernel.
  - AV accumulators split into per-512-column half tiles (4x 1-bank PSUM);
    the q<512 half is evicted + normalized while the q>=512 half still
    accumulates; the rank-1 denominator broadcast targets rows 0..63 of
    the just-evicted AV bank (no PSUM pool contention) and runs in bf16.
  - V eviction on ScalarE, keeping DVE off the scores->AV critical path.
  - output projection split into 16 (st, nE) 1-bank chunks; 6 chunks
    pre-accumulate ko 0..6 during pair 7 on freed banks; y written with
    one full-row [128,1024] DMA per st so the output drain overlaps the
    proj matmuls.

Layout (per core, S=1024, D=1024, H=16, hd=64): as baseline — q,k transposed
[e,s] per head-pair; v natural [s,e] with a ones column per head (AV PSUM row
64 = softmax denominator); scoresT [sk,sq]; exp on ACT (scale 1/8 folded);
per-head normalization via fp32 approx-reciprocal + PE rank-1 broadcast.
"""

import numpy as np

B, S, D, H = 8, 1024, 1024, 16
HD = D // H          # 64
P = 128
NCORES = 8
KO = D // P          # 8 contraction tiles over d
ST = S // P          # 8 s-tiles
NPAIRS = H // 2      # 8 head pairs

_CACHE = {}
TRACE = False        # set by test harness to collect an NTFF profile


def _score_chunks(w):
    # pieces <=512 (PSUM bank limit); bf16 streams full-rate at any width
    table = {1024: [512, 512], 896: [512, 384], 768: [512, 256],
             640: [384, 256], 512: [512], 384: [384], 256: [256], 128: [128]}
    return table[w]


def _build():
    import concourse.tile as tile
    from concourse import bacc, mybir

    BF = mybir.dt.bfloat16
    F32R = mybir.dt.float32r
    F32 = mybir.dt.float32
    AF = mybir.ActivationFunctionType

    nc = bacc.Bacc("TRN2", target_bir_lowering=False, debug=False,
                   num_devices=NCORES)
    xT_d = nc.dram_tensor("xT", [D, S], BF, kind="ExternalInput").ap()
    # host-interleaved: col block 256*j holds [q m-tile j | k m-tile j]
    wqkT_d = nc.dram_tensor("wqkT", [D, 2 * D], BF, kind="ExternalInput").ap()
    wvT_d = nc.dram_tensor("wvT", [D, D], BF, kind="ExternalInput").ap()
    wpT_d = nc.dram_tensor("wpT", [D, D], BF, kind="ExternalInput").ap()
    bqk_d = nc.dram_tensor("bqk", [2 * D], F32, kind="ExternalInput").ap()
    beffb_d = nc.dram_tensor("beffb", [P, D], F32, kind="ExternalInput").ap()
    umask_d = nc.dram_tensor("umask", [P, P], BF, kind="ExternalInput").ap()
    y_d = nc.dram_tensor("y", [S, D], F32, kind="ExternalOutput").ap()

    wqkT_v = wqkT_d.rearrange("(ko p) e -> p ko e", p=P)
    wvT_v = wvT_d.rearrange("(ko p) e -> p ko e", p=P)
    wpT_v = wpT_d.rearrange("(ko p) e -> p ko e", p=P)
    xT_v = xT_d.rearrange("(ko p) s -> p ko s", p=P)

    with tile.TileContext(nc) as tc:
        with (
            tc.tile_pool(name="bigio", bufs=1) as bigio,
            tc.tile_pool(name="qkp", bufs=3) as qkp,
            tc.tile_pool(name="vp", bufs=1) as vpool,
            tc.tile_pool(name="attn", bufs=12) as attnp,
            tc.tile_pool(name="rt", bufs=2) as rtp,
            tc.tile_pool(name="todd", bufs=2) as toddp,
            tc.tile_pool(name="ystg", bufs=2) as ystgp,
            tc.tile_pool(name="avsb", bufs=2) as avsbp,
            tc.tile_pool(name="cst", bufs=1) as cst,
            tc.tile_pool(name="psS", bufs=3, space="PSUM") as psS,
            tc.tile_pool(name="psQ", bufs=1, space="PSUM") as psQ,
            tc.tile_pool(name="psAV", bufs=1, space="PSUM") as psAV,
        ):
            # ---------- small constants ----------
            umask = cst.tile([P, P], BF)
            bqk_sb = cst.tile([P, 2 * D // P], F32)

            # ---------- big SBUF residents (DMA priority order) ----------
            # per-pair qk weights [P, KO, 256]: cols 0:128 = q m-tile j,
            # 128:256 = k m-tile j (host interleaved)
            wqk_t = [bigio.tile([P, KO, 2 * P], BF, tag=f"wqk{j}",
                                name=f"wqk{j}")
                     for j in range(NPAIRS)]
            xT = bigio.tile([P, KO, S], BF, tag="xT")
            wv_t = [bigio.tile([P, KO, 512], BF, tag=f"wv{e}", name=f"wv{e}")
                    for e in range(2)]
            wp_t = [bigio.tile([P, KO, 512], BF, tag=f"wp{e}", name=f"wp{e}")
                    for e in range(2)]

            # priority order, few LARGE DMAs (each DMA_DIRECT2D costs ~0.5us
            # of issue time on the Sync queue): pair-0 weights + x quarters
            # first so the first chains unblock after ~1.5MB of DMA; then
            # wv0 (v chains run during pair 0); later pairs at a relaxed
            # pace.
            nc.sync.dma_start(wqk_t[0][:, :, 0:P], wqkT_v[:, :, 0:P])
            nc.sync.dma_start(xT[:, 0:4, 0:512], xT_v[:, 0:4, 0:512])
            nc.sync.dma_start(wqk_t[0][:, :, P:2 * P], wqkT_v[:, :, P:2 * P])
            nc.sync.dma_start(xT[:, 4:8, 0:512], xT_v[:, 4:8, 0:512])
            nc.sync.dma_start(umask[:], umask_d)
            nc.sync.dma_start(bqk_sb[:], bqk_d.rearrange("(m p) -> p m", p=P))
            nc.sync.dma_start(wqk_t[1][:], wqkT_v[:, :, 2 * P:4 * P])
            nc.sync.dma_start(xT[:, 0:4, 512:1024], xT_v[:, 0:4, 512:1024])
            nc.sync.dma_start(xT[:, 4:8, 512:1024], xT_v[:, 4:8, 512:1024])
            nc.sync.dma_start(wv_t[0][:], wvT_v[:, :, 0:512])
            for j in (2, 3):
                nc.sync.dma_start(wqk_t[j][:],
                                  wqkT_v[:, :, 2 * P * j:2 * P * (j + 1)])
            nc.sync.dma_start(wv_t[1][:], wvT_v[:, :, 512:1024])
            for j in (4, 5, 6, 7):
                nc.sync.dma_start(wqk_t[j][:],
                                  wqkT_v[:, :, 2 * P * j:2 * P * (j + 1)])
            for e in range(2):
                nc.sync.dma_start(wp_t[e][:],
                                  wpT_v[:, :, 512 * e:512 * (e + 1)])
            beffb = cst.tile([P, D], F32)
            nc.sync.dma_start(beffb[:], beffb_d)

            # ---------- derived constants / staging ----------
            # HAM pre-warm: ~35 dense junk matmuls during the DMA prologue
            # lift the PE clock gate to 8/8 before the first real chains
            junk = cst.tile([P, 512], BF, name="junk")
            nc.vector.memset(junk[:], 0.0)
            jps = psQ.tile([P, 512], F32, tag="psq", name="warmps")
            for _ in range(20):
                nc.tensor.matmul(jps[:], junk[:, 0:P], junk[:],
                                 start=True, stop=True)

            onecol = cst.tile([P, 1], BF)
            nc.vector.memset(onecol[:], 1.0)
            # bf16 ones row: bf16 rank-1 matmuls stream at full rate (f32r
            # moving streams at half rate and disables FWL on the next LDW)
            ones_r = cst.tile([65, 64], BF)
            nc.vector.memset(ones_r[64:65, :], 1.0)

            outT = bigio.tile([P, KO, S], BF, tag="outT")
            v_sb = vpool.tile([P, ST, H * (HD + 1)], BF)
            v_hview = v_sb[:].rearrange("p st (h c) -> p st h c", c=HD + 1)
            nc.vector.tensor_copy(
                v_hview[:, :, :, HD:HD + 1],
                onecol[:, None, None, :].broadcast_to([P, ST, H, 1]))

            qk_tiles = {}    # j -> [128, 2, S] tile (0=q, 1=k)

            # ---------- QKV work chains (each: 8 matmuls, 1 PSUM bank) ----
            def qk_chain(j, part, nn, pool_tag=None):   # part 0=q, 1=k
                def go():
                    t = qk_tiles[j]
                    pool, tag = pool_tag or (psQ, "psq")
                    ps = pool.tile([P, 512], F32, tag=tag,
                                   name=f"qkps{j}_{part}_{nn}")
                    for ko in range(KO):
                        nc.tensor.matmul(
                            ps[:], wqk_t[j][:, ko, part * P:(part + 1) * P],
                            xT[:, ko, nn * 512:(nn + 1) * 512],
                            start=(ko == 0), stop=(ko == KO - 1))
                    mi = 2 * j + part
                    nc.vector.tensor_scalar_add(
                        t[:, part, nn * 512:(nn + 1) * 512], ps[:],
                        bqk_sb[:, mi:mi + 1])
                return go

            def qk_alloc(j, pool_tags=None):
                qk_tiles[j] = qkp.tile([P, 2, S], BF, tag="qkt",
                                       name=f"qk{j}")
                pts = pool_tags or [None] * 4
                # nn=0 halves first: unblocks the first scores chunks
                return [qk_chain(j, part, nn, pts[2 * nn + part])
                        for nn in (0, 1) for part in (0, 1)]

            def v_chain(st, nE, pool_tag=None):
                def go():
                    pool, tag = pool_tag or (psQ, "psq")
                    ps = pool.tile([P, 512], F32, tag=tag,
                                   name=f"vps{nE}_{st}")
                    for ko in range(KO):
                        nc.tensor.matmul(
                            ps[:], xT[:, ko, st * P:(st + 1) * P],
                            wv_t[nE][:, ko, :],
                            start=(ko == 0), stop=(ko == KO - 1))
                    # eviction on ScalarE keeps DVE off the critical path
                    nc.scalar.activation(
                        v_hview[:, st, 8 * nE:8 * (nE + 1), 0:HD],
                        ps[:].rearrange("p (h c) -> p h c", c=HD),
                        AF.Copy)
                return go

            # ---------- attention ----------
            pend = {}

            def scores_exp(j, m):
                # two heads' matmuls adjacent per chunk: they run
                # concurrently on PE row groups 0-63 / 64-127
                qk_t = qk_tiles[j]
                w = S - m * P
                ats = []
                for hb, base in ((0, 0), (1, 64)):
                    at = attnp.tile([P, S], BF, tag="at",
                                    name=f"at{j}_{hb}_{m}")
                    pend[(j, hb, m)] = at
                    ats.append(at)
                off = m * P
                for cw in _score_chunks(w):
                    pss = []
                    for hb, base in ((0, 0), (1, 64)):
                        ps = psS.tile([P, 512], F32, tag="ps",
                                      name=f"sps{j}_{hb}_{m}")
                        nc.tensor.matmul(
                            ps[:, 0:cw],
                            qk_t[base:base + 64, 1, m * P:(m + 1) * P],
                            qk_t[base:base + 64, 0, off:off + cw],
                            start=True, stop=True)
                        pss.append(ps)
                    for hb in (0, 1):
                        nc.scalar.activation(
                            ats[hb][:, off:off + cw], pss[hb][:, 0:cw],
                            AF.Exp, scale=0.125)
                    off += cw
                for hb in (0, 1):
                    nc.vector.tensor_mul(
                        ats[hb][:, m * P:(m + 1) * P],
                        ats[hb][:, m * P:(m + 1) * P], umask[:])

            def av_alloc(j):
                # per-half accumulators: [hb][half], 1 PSUM bank each.
                # declared [128,512]: rows 0..64 = AV out + den; rows 0..63
                # are re-used post-eviction as the rank-1 broadcast target.
                pend[f"ps{j}"] = [
                    [psAV.tile([P, 512], F32, tag=f"av{hb}_{n}",
                               name=f"av{j}_{hb}_{n}") for n in range(2)]
                    for hb in range(2)]

            def av_m(j, m):
                # narrowed to the causally-nonzero range of each 512 chunk
                st8 = pend[f"ps{j}"]
                for hb in (0, 1):
                    h = 2 * j + hb
                    at = pend[(j, hb, m)]
                    for n in range(2):
                        lo = max(n * 512, m * P)
                        hi = (n + 1) * 512
                        if lo >= hi:
                            continue
                        nc.tensor.matmul(
                            st8[hb][n][0:65, lo - n * 512:hi - n * 512],
                            v_sb[:, m, h * (HD + 1):(h + 1) * (HD + 1)],
                            at[:, lo:hi],
                            start=(m == 0), stop=(m == 4 * n + 3),
                            skip_group_check=True)

            def rb_prep(j, n):
                # evict the q-half [65,512] AV accumulator and take the
                # fp32 approx-reciprocal of its den row (all DVE, no PSUM
                # bank held)
                for hb in (0, 1):
                    avc = avsbp.tile([65, 512], F32, tag=f"avc{hb}_{n}",
                                     name=f"avc{j}_{hb}_{n}")
                    nc.vector.tensor_copy(avc[:],
                                          pend[f"ps{j}"][hb][n][0:65, :])
                    rt = rtp.tile([65, 512], BF, tag=f"rt{hb}_{n}")
                    rt32 = rtp.tile([65, 512], F32, tag=f"rt32{n}", bufs=1)
                    # custom-DVE op misbehaves on single-partition APs on HW:
                    # run over all 65 rows, consume only the den row (64)
                    nc.vector.reciprocal_approx_fast(rt32[:], avc[:])
                    nc.vector.tensor_copy(rt[64:65, :], rt32[64:65, :])
                    pend[f"rb{j}_{hb}_{n}"] = (avc, rt)

            def rb_fin(j, n):
                # rank-1 broadcast of the reciprocal into rows 0..63 of the
                # (already evicted) AV PSUM bank, then normalize into outT
                for hb in (0, 1):
                    avc, rt = pend.pop(f"rb{j}_{hb}_{n}")
                    rps = pend[f"ps{j}"][hb][n]
                    nc.tensor.matmul(
                        rps[0:64, :], ones_r[64:65, :], rt[64:65, :],
                        start=True, stop=True)
                    if hb == 0:
                        nc.vector.tensor_mul(
                            outT[0:64, j, n * 512:(n + 1) * 512],
                            avc[0:64, :], rps[0:64, :])
                    else:
                        # DVE lanes cannot shift partitions: multiply to
                        # SBUF tmp, DMA-shift rows 0..63 -> 64..127
                        tmp = toddp.tile([64, 512], BF, tag="todd")
                        nc.vector.tensor_mul(tmp[:], avc[0:64, :],
                                             rps[0:64, :])
                        nc.sync.dma_start(
                            outT[64:128, j, n * 512:(n + 1) * 512], tmp[:])

            # ---------- output projection chunks ----------
            proj_ps = {}

            def proj_mm(st, nE, kos, pool, tag):
                if (st, nE) not in proj_ps:
                    proj_ps[(st, nE)] = pool.tile(
                        [P, 512], F32, tag=tag, name=f"yps{st}_{nE}")
                ps = proj_ps[(st, nE)]
                for ko in kos:
                    nc.tensor.matmul(
                        ps[:], outT[:, ko, st * P:(st + 1) * P],
                        wp_t[nE][:, ko, :],
                        start=(ko == 0), stop=(ko == KO - 1))

            def proj_evict(st, ystg_halves):
                for nE in range(2):
                    nc.vector.tensor_add(
                        ystg_halves[nE], proj_ps[(st, nE)][:],
                        beffb[:, nE * 512:(nE + 1) * 512])

            # ---------- interleaved emission ----------
            # prologue: qk for pair 0 and the first two v chains
            # prologue chains spread over the (still idle) AV banks so the
            # DMA-paced ramp isn't serialized through the single psQ bank
            for c in qk_alloc(0, [(psQ, "psq"), (psAV, "av0_0"),
                                  (psAV, "av0_1"), (psAV, "av1_0")]):
                c()
            v_chain(0, 0, (psAV, "av1_1"))()
            v_chain(1, 0, (psQ, "psq"))()
            vwork0 = [v_chain(st, 0) for st in range(2, ST)]
            vwork1 = [v_chain(st, 1) for st in range(ST)]

            # fill units per pair: pair j computes pair j+1's chains except
            # its k nn=1 chain, which runs early IN pair j+1 itself (the
            # nn=1 k-stationary is only needed from scores step 4) — this
            # keeps every pair boundary, including pair 7's, PE-dense;
            # v half-0 spread through pair 0 (needed by pair-0 AV), v
            # half-1 over pairs 1-3 (needed by pair 4)
            qku = {j: qk_alloc(j) for j in range(1, NPAIRS)}
            fill = {j: [] for j in range(NPAIRS)}
            fill[0] = vwork0 + qku[1]
            for j, nv in ((1, 2), (2, 2), (3, 2), (4, 2)):
                fill[j] = [vwork1.pop(0) for _ in range(nv)]
            for j in range(1, NPAIRS - 1):
                fill[j] = fill[j] + qku[j + 1]

            # spread sparse fill toward late/boundary steps (the PE dips
            # that re-throttle HAM happen at pair boundaries)
            POP_ORDER = [1, 3, 5, 7, 0, 2, 4, 6]
            for j in range(NPAIRS):
                work = list(fill[j])
                psteps = sorted(POP_ORDER[:min(len(work), ST)])
                for m in range(ST):
                    # AV of step m-2 is ready (its at-tile was exp'd two
                    # steps ago): emit it AHEAD of the exp-gated scores so
                    # the in-order TE queue never stalls behind a waiting
                    # scores matmul
                    if m == 0:
                        if j > 0:
                            rb_fin(j - 1, 1)
                        av_alloc(j)
                    if m >= 2:
                        av_m(j, m - 2)
                    scores_exp(j, m)
                    if m == 6:
                        rb_prep(j, 0)
                    if m == 7:
                        rb_fin(j, 0)
                    if work and (m in psteps or len(work) > ST - m):
                        work.pop(0)()
                av_m(j, ST - 2)
                while work:
                    work.pop(0)()
                av_m(j, ST - 1)
                rb_prep(j, 1)
                if j == NPAIRS - 1:
                    # finish pair 7's normalization immediately, then
                    # prestart eight proj chunks (ko 0..6) on the freed
                    # psS/psQ/av banks while pair 7's tail drains
                    rb_fin(j, 1)
                    proj_mm(0, 0, range(7), psS, "ps")
                    proj_mm(0, 1, range(7), psS, "ps")
                    proj_mm(1, 0, range(7), psS, "ps")
                    proj_mm(1, 1, range(7), psQ, "psq")
                    proj_mm(2, 0, range(7), psAV, "av0_0")
                    proj_mm(2, 1, range(7), psAV, "av1_0")
                    proj_mm(3, 0, range(7), psAV, "av0_1")
                    proj_mm(3, 1, range(7), psAV, "av1_1")

            # ---------- output projection ----------
            slots = [(psQ, "psq"), (psAV, "av0_0"),
                     (psAV, "av0_1"), (psAV, "av1_0"), (psAV, "av1_1"),
                     (psS, "ps")]
            si = 0
            # finish the prestarted st=0..3 chunks, then st 4..7
            for st in range(ST):
                ystg = ystgp.tile([P, D], F32, tag="ystg", name=f"ystg{st}")
                for nE in range(2):
                    if st <= 3:
                        proj_mm(st, nE, range(7, 8), psS, "ps")
                    else:
                        pool, tag = slots[si % len(slots)]
                        proj_mm(st, nE, range(KO), pool, tag)
                        si += 1
                proj_evict(st, [ystg[:, 0:512], ystg[:, 512:1024]])
                nc.sync.dma_start(y_d[st * P:(st + 1) * P, :], ystg[:])

    nc.compile()
    return nc


def kernel(x, w_attn, b_attn, w_proj, b_proj):
    import concourse.bass_utils as bass_utils
    import ml_dtypes

    if "nc" not in _CACHE:
        _CACHE["nc"] = _build()
    nc = _CACHE["nc"]

    bf16 = ml_dtypes.bfloat16
    x = np.asarray(x, dtype=np.float32)
    w_attn = np.asarray(w_attn, dtype=np.float32)
    b_attn = np.asarray(b_attn, dtype=np.float32)
    w_proj = np.asarray(w_proj, dtype=np.float32)
    b_proj = np.asarray(b_proj, dtype=np.float32)

    xT = np.ascontiguousarray(
        np.transpose(x, (0, 2, 1))).astype(bf16)                 # [B, D, S]
    # interleave q/k per head pair: col block 256*j = [q m-tile j | k m-tile j]
    wqT = w_attn[:D].T.reshape(D, NPAIRS, P)                     # [D, 8, 128]
    wkT = w_attn[D:2 * D].T.reshape(D, NPAIRS, P)
    wqkT = np.ascontiguousarray(
        np.concatenate([wqT[:, :, None, :], wkT[:, :, None, :]], axis=2)
        .reshape(D, 2 * D)).astype(bf16)                         # [D, 2D]
    bq = b_attn[:D].reshape(NPAIRS, P)
    bk = b_attn[D:2 * D].reshape(NPAIRS, P)
    bqk = np.ascontiguousarray(
        np.concatenate([bq[:, None, :], bk[:, None, :]], axis=1).reshape(-1))
    wvT = np.ascontiguousarray(w_attn[2 * D:].T).astype(bf16)    # [D, D]
    wpT = np.ascontiguousarray(w_proj.T).astype(bf16)            # [D, D]
    bv = b_attn[2 * D:]
    beff = (b_proj.astype(np.float64)
            + w_proj.astype(np.float64) @ bv.astype(np.float64)
            ).astype(np.float32)
    beffb = np.ascontiguousarray(np.broadcast_to(beff, (P, D)))
    umask = np.triu(np.ones((P, P), dtype=np.float32)).astype(bf16)

    in_maps = [
        dict(xT=xT[b], wqkT=wqkT, wvT=wvT, wpT=wpT, bqk=bqk, beffb=beffb,
             umask=umask)
        for b in range(B)
    ]
    res = bass_utils.run_bass_kernel_spmd(
        nc, in_maps, core_ids=list(range(NCORES)), trace=TRACE)
    if TRACE:
        _CACHE["exec_time_ns"] = res.exec_time_ns
        _CACHE["trace"] = res.instructions_and_trace
    return np.stack([res.results[b]["y"] for b in range(B)], axis=0)


# revision 46
# speedup vs baseline: 1.1870x; 1.1870x over previous
"""Causal self-attention on 8 TRN2 NeuronCores, batch-data-parallel (one batch
element per core).

Schedule (evolved by trace analysis from a 271us baseline to ~235us):
  - wqk host-interleaved per head pair ([q_j | k_j] 256-col blocks), few
    large priority-ordered DMAs (each DMA_DIRECT2D costs ~0.5us of Sync
    issue time): the first QK chain depends on ~0.75MB instead of 6MB, so
    the PE starts at ~9us instead of 20us; 16 junk warm-up matmuls during
    the DMA prologue lift the HAM clock gate (1.2->2.4GHz) early.
  - QKV fill chains rebalanced: pair j computes pair j+1's qk chains, v
    chains spread over pairs 0-3, pops spread to odd steps so pairs 4-6
    stay PE-dense and HAM never re-throttles mid-kernel.
  - AV accumulators split into per-512-column half tiles (4x 1-bank PSUM);
    the q<512 half is evicted + normalized while the q>=512 half still
    accumulates; the rank-1 denominator broadcast targets rows 0..63 of
    the just-evicted AV bank (no PSUM pool contention) and runs in bf16.
  - V eviction on ScalarE, keeping DVE off the scores->AV critical path.
  - output projection split into 16 (st, nE) 1-bank chunks; 6 chunks
    pre-accumulate ko 0..6 during pair 7 on freed banks; y written with
    one full-row [128,1024] DMA per st so the output drain overlaps the
    proj matmuls.

Layout (per core, S=1024, D=1024, H=16, hd=64): as baseline — q,k transposed
[e,s] per head-pair; v natural [s,e] with a ones column per head (AV PSUM row
64 = softmax denominator); scoresT [sk,sq]; exp on ACT (scale 1/8 folded);
per-head normalization via fp32 approx-reciprocal + PE rank-1 broadcast.
"""

import numpy as np

B, S, D, H = 8, 1024, 1024, 16
HD = D // H          # 64
P = 128
NCORES = 8
KO = D // P          # 8 contraction tiles over d
ST = S // P          # 8 s-tiles
NPAIRS = H // 2      # 8 head pairs

_CACHE = {}
TRACE = False        # set by test harness to collect an NTFF profile


def _score_chunks(w):
    # pieces <=512 (PSUM bank limit); bf16 streams full-rate at any width
    table = {1024: [512, 512], 896: [512, 384], 768: [512, 256],
             640: [384, 256], 512: [512], 384: [384], 256: [256], 128: [128]}
    return table[w]


def _build():
    import concourse.tile as tile
    from concourse import bacc, mybir

    BF = mybir.dt.bfloat16
    F32R = mybir.dt.float32r
    F32 = mybir.dt.float32
    AF = mybir.ActivationFunctionType

    nc = bacc.Bacc("TRN2", target_bir_lowering=False, debug=False,
                   num_devices=NCORES)
    xT_d = nc.dram_tensor("xT", [D, S], BF, kind="ExternalInput").ap()
    # host-interleaved: col block 256*j holds [q m-tile j | k m-tile j]
    wqkT_d = nc.dram_tensor("wqkT", [D, 2 * D], BF, kind="ExternalInput").ap()
    wvT_d = nc.dram_tensor("wvT", [D, D], BF, kind="ExternalInput").ap()
    wpT_d = nc.dram_tensor("wpT", [D, D], BF, kind="ExternalInput").ap()
    bqk_d = nc.dram_tensor("bqk", [2 * D], F32, kind="ExternalInput").ap()
    beffb_d = nc.dram_tensor("beffb", [P, D], F32, kind="ExternalInput").ap()
    umask_d = nc.dram_tensor("umask", [P, P], BF, kind="ExternalInput").ap()
    y_d = nc.dram_tensor("y", [S, D], F32, kind="ExternalOutput").ap()

    wqkT_v = wqkT_d.rearrange("(ko p) e -> p ko e", p=P)
    wvT_v = wvT_d.rearrange("(ko p) e -> p ko e", p=P)
    wpT_v = wpT_d.rearrange("(ko p) e -> p ko e", p=P)
    xT_v = xT_d.rearrange("(ko p) s -> p ko s", p=P)

    with tile.TileContext(nc) as tc:
        with (
            tc.tile_pool(name="bigio", bufs=1) as bigio,
            tc.tile_pool(name="qkp", bufs=3) as qkp,
            tc.tile_pool(name="vp", bufs=1) as vpool,
            tc.tile_pool(name="attn", bufs=8) as attnp,
            tc.tile_pool(name="rt", bufs=2) as rtp,
            tc.tile_pool(name="todd", bufs=2) as toddp,
            tc.tile_pool(name="ystg", bufs=2) as ystgp,
            tc.tile_pool(name="avsb", bufs=2) as avsbp,
            tc.tile_pool(name="cst", bufs=1) as cst,
            tc.tile_pool(name="psS", bufs=3, space="PSUM") as psS,
            tc.tile_pool(name="psQ", bufs=1, space="PSUM") as psQ,
            tc.tile_pool(name="psAV", bufs=1, space="PSUM") as psAV,
        ):
            # ---------- small constants ----------
            umask = cst.tile([P, P], BF)
            bqk_sb = cst.tile([P, 2 * D // P], F32)

            # ---------- big SBUF residents (DMA priority order) ----------
            # per-pair qk weights [P, KO, 256]: cols 0:128 = q m-tile j,
            # 128:256 = k m-tile j (host interleaved)
            wqk_t = [bigio.tile([P, KO, 2 * P], BF, tag=f"wqk{j}",
                                name=f"wqk{j}")
                     for j in range(NPAIRS)]
            xT = bigio.tile([P, KO, S], BF, tag="xT")
            wv_t = [bigio.tile([P, KO, 512], BF, tag=f"wv{e}", name=f"wv{e}")
                    for e in range(2)]
            wp_t = [bigio.tile([P, KO, 512], BF, tag=f"wp{e}", name=f"wp{e}")
                    for e in range(2)]

            # priority order, few LARGE DMAs (each DMA_DIRECT2D costs ~0.5us
            # of issue time on the Sync queue): pair-0 weights + x quarters
            # first so the first chains unblock after ~1.5MB of DMA; then
            # wv0 (v chains run during pair 0); later pairs at a relaxed
            # pace.
            nc.sync.dma_start(wqk_t[0][:, :, 0:P], wqkT_v[:, :, 0:P])
            nc.sync.dma_start(xT[:, 0:4, 0:512], xT_v[:, 0:4, 0:512])
            nc.sync.dma_start(wqk_t[0][:, :, P:2 * P], wqkT_v[:, :, P:2 * P])
            nc.sync.dma_start(xT[:, 4:8, 0:512], xT_v[:, 4:8, 0:512])
            nc.sync.dma_start(umask[:], umask_d)
            nc.sync.dma_start(bqk_sb[:], bqk_d.rearrange("(m p) -> p m", p=P))
            nc.sync.dma_start(wqk_t[1][:], wqkT_v[:, :, 2 * P:4 * P])
            nc.sync.dma_start(xT[:, 0:4, 512:1024], xT_v[:, 0:4, 512:1024])
            nc.sync.dma_start(xT[:, 4:8, 512:1024], xT_v[:, 4:8, 512:1024])
            nc.sync.dma_start(wv_t[0][:], wvT_v[:, :, 0:512])
            for j in (2, 3):
                nc.sync.dma_start(wqk_t[j][:],
                                  wqkT_v[:, :, 2 * P * j:2 * P * (j + 1)])
            nc.sync.dma_start(wv_t[1][:], wvT_v[:, :, 512:1024])
            for j in (4, 5, 6, 7):
                nc.sync.dma_start(wqk_t[j][:],
                                  wqkT_v[:, :, 2 * P * j:2 * P * (j + 1)])
            for e in range(2):
                nc.sync.dma_start(wp_t[e][:],
                                  wpT_v[:, :, 512 * e:512 * (e + 1)])
            beffb = cst.tile([P, D], F32)
            nc.sync.dma_start(beffb[:], beffb_d)

            # ---------- derived constants / staging ----------
            # HAM pre-warm: ~35 dense junk matmuls during the DMA prologue
            # lift the PE clock gate to 8/8 before the first real chains
            junk = cst.tile([P, 512], BF, name="junk")
            nc.vector.memset(junk[:], 0.0)
            jps = psQ.tile([P, 512], F32, tag="psq", name="warmps")
            for _ in range(26):
                nc.tensor.matmul(jps[:], junk[:, 0:P], junk[:],
                                 start=True, stop=True)

            onecol = cst.tile([P, 1], BF)
            nc.vector.memset(onecol[:], 1.0)
            # bf16 ones row: bf16 rank-1 matmuls stream at full rate (f32r
            # moving streams at half rate and disables FWL on the next LDW)
            ones_r = cst.tile([65, 64], BF)
            nc.vector.memset(ones_r[64:65, :], 1.0)

            outT = bigio.tile([P, KO, S], BF, tag="outT")
            v_sb = vpool.tile([P, ST, H * (HD + 1)], BF)
            v_hview = v_sb[:].rearrange("p st (h c) -> p st h c", c=HD + 1)
            nc.vector.tensor_copy(
                v_hview[:, :, :, HD:HD + 1],
                onecol[:, None, None, :].broadcast_to([P, ST, H, 1]))

            qk_tiles = {}    # j -> [128, 2, S] tile (0=q, 1=k)

            # ---------- QKV work chains (each: 8 matmuls, 1 PSUM bank) ----
            def qk_chain(j, part, nn, pool_tag=None):   # part 0=q, 1=k
                def go():
                    t = qk_tiles[j]
                    pool, tag = pool_tag or (psQ, "psq")
                    ps = pool.tile([P, 512], F32, tag=tag,
                                   name=f"qkps{j}_{part}_{nn}")
                    for ko in range(KO):
                        nc.tensor.matmul(
                            ps[:], wqk_t[j][:, ko, part * P:(part + 1) * P],
                            xT[:, ko, nn * 512:(nn + 1) * 512],
                            start=(ko == 0), stop=(ko == KO - 1))
                    mi = 2 * j + part
                    nc.vector.tensor_scalar_add(
                        t[:, part, nn * 512:(nn + 1) * 512], ps[:],
                        bqk_sb[:, mi:mi + 1])
                return go

            def qk_alloc(j, pool_tags=None):
                qk_tiles[j] = qkp.tile([P, 2, S], BF, tag="qkt",
                                       name=f"qk{j}")
                pts = pool_tags or [None] * 4
                # nn=0 halves first: unblocks the first scores chunks
                return [qk_chain(j, part, nn, pts[2 * nn + part])
                        for nn in (0, 1) for part in (0, 1)]

            def v_chain(st, nE, pool_tag=None):
                def go():
                    pool, tag = pool_tag or (psQ, "psq")
                    ps = pool.tile([P, 512], F32, tag=tag,
                                   name=f"vps{nE}_{st}")
                    for ko in range(KO):
                        nc.tensor.matmul(
                            ps[:], xT[:, ko, st * P:(st + 1) * P],
                            wv_t[nE][:, ko, :],
                            start=(ko == 0), stop=(ko == KO - 1))
                    # eviction on ScalarE keeps DVE off the critical path
                    nc.scalar.activation(
                        v_hview[:, st, 8 * nE:8 * (nE + 1), 0:HD],
                        ps[:].rearrange("p (h c) -> p h c", c=HD),
                        AF.Copy)
                return go

            # ---------- attention ----------
            pend = {}

            def scores_exp(j, m):
                # two heads' matmuls adjacent per chunk: they run
                # concurrently on PE row groups 0-63 / 64-127
                qk_t = qk_tiles[j]
                w = S - m * P
                ats = []
                for hb, base in ((0, 0), (1, 64)):
                    at = attnp.tile([P, S], BF, tag="at",
                                    name=f"at{j}_{hb}_{m}")
                    pend[(j, hb, m)] = at
                    ats.append(at)
                off = m * P
                for cw in _score_chunks(w):
                    pss = []
                    for hb, base in ((0, 0), (1, 64)):
                        ps = psS.tile([P, 512], F32, tag="ps",
                                      name=f"sps{j}_{hb}_{m}")
                        nc.tensor.matmul(
                            ps[:, 0:cw],
                            qk_t[base:base + 64, 1, m * P:(m + 1) * P],
                            qk_t[base:base + 64, 0, off:off + cw],
                            start=True, stop=True)
                        pss.append(ps)
                    for hb in (0, 1):
                        nc.scalar.activation(
                            ats[hb][:, off:off + cw], pss[hb][:, 0:cw],
                            AF.Exp, scale=0.125)
                    off += cw
                for hb in (0, 1):
                    nc.vector.tensor_mul(
                        ats[hb][:, m * P:(m + 1) * P],
                        ats[hb][:, m * P:(m + 1) * P], umask[:])

            def av_alloc(j):
                # per-half accumulators: [hb][half], 1 PSUM bank each.
                # declared [128,512]: rows 0..64 = AV out + den; rows 0..63
                # are re-used post-eviction as the rank-1 broadcast target.
                pend[f"ps{j}"] = [
                    [psAV.tile([P, 512], F32, tag=f"av{hb}_{n}",
                               name=f"av{j}_{hb}_{n}") for n in range(2)]
                    for hb in range(2)]

            def av_m(j, m):
                # narrowed to the causally-nonzero range of each 512 chunk
                st8 = pend[f"ps{j}"]
                for hb in (0, 1):
                    h = 2 * j + hb
                    at = pend[(j, hb, m)]
                    for n in range(2):
                        lo = max(n * 512, m * P)
                        hi = (n + 1) * 512
                        if lo >= hi:
                            continue
                        nc.tensor.matmul(
                            st8[hb][n][0:65, lo - n * 512:hi - n * 512],
                            v_sb[:, m, h * (HD + 1):(h + 1) * (HD + 1)],
                            at[:, lo:hi],
                            start=(m == 0), stop=(m == 4 * n + 3),
                            skip_group_check=True)

            def rb_prep(j, n):
                # evict the q-half [65,512] AV accumulator and take the
                # fp32 approx-reciprocal of its den row (all DVE, no PSUM
                # bank held)
                for hb in (0, 1):
                    avc = avsbp.tile([65, 512], F32, tag=f"avc{hb}_{n}",
                                     name=f"avc{j}_{hb}_{n}")
                    nc.vector.tensor_copy(avc[:],
                                          pend[f"ps{j}"][hb][n][0:65, :])
                    rt = rtp.tile([65, 512], BF, tag=f"rt{hb}_{n}")
                    rt32 = rtp.tile([65, 512], F32, tag=f"rt32{n}", bufs=1)
                    # custom-DVE op misbehaves on single-partition APs on HW:
                    # run over all 65 rows, consume only the den row (64)
                    nc.vector.reciprocal_approx_fast(rt32[:], avc[:])
                    nc.vector.tensor_copy(rt[64:65, :], rt32[64:65, :])
                    pend[f"rb{j}_{hb}_{n}"] = (avc, rt)

            def rb_fin(j, n):
                # rank-1 broadcast of the reciprocal into rows 0..63 of the
                # (already evicted) AV PSUM bank, then normalize into outT
                for hb in (0, 1):
                    avc, rt = pend.pop(f"rb{j}_{hb}_{n}")
                    rps = pend[f"ps{j}"][hb][n]
                    nc.tensor.matmul(
                        rps[0:64, :], ones_r[64:65, :], rt[64:65, :],
                        start=True, stop=True)
                    if hb == 0:
                        nc.vector.tensor_mul(
                            outT[0:64, j, n * 512:(n + 1) * 512],
                            avc[0:64, :], rps[0:64, :])
                    else:
                        # DVE lanes cannot shift partitions: multiply to
                        # SBUF tmp, DMA-shift rows 0..63 -> 64..127
                        tmp = toddp.tile([64, 512], BF, tag="todd")
                        nc.vector.tensor_mul(tmp[:], avc[0:64, :],
                                             rps[0:64, :])
                        nc.sync.dma_start(
                            outT[64:128, j, n * 512:(n + 1) * 512], tmp[:])

            # ---------- output projection chunks ----------
            proj_ps = {}

            def proj_mm(st, nE, kos, pool, tag):
                if (st, nE) not in proj_ps:
                    proj_ps[(st, nE)] = pool.tile(
                        [P, 512], F32, tag=tag, name=f"yps{st}_{nE}")
                ps = proj_ps[(st, nE)]
                for ko in kos:
                    nc.tensor.matmul(
                        ps[:], outT[:, ko, st * P:(st + 1) * P],
                        wp_t[nE][:, ko, :],
                        start=(ko == 0), stop=(ko == KO - 1))

            def proj_evict(st, ystg_halves):
                for nE in range(2):
                    nc.vector.tensor_add(
                        ystg_halves[nE], proj_ps[(st, nE)][:],
                        beffb[:, nE * 512:(nE + 1) * 512])

            # ---------- interleaved emission ----------
            # prologue: qk for pair 0 and the first two v chains
            # prologue chains spread over the (still idle) AV banks so the
            # DMA-paced ramp isn't serialized through the single psQ bank
            for c in qk_alloc(0, [(psQ, "psq"), (psAV, "av0_0"),
                                  (psAV, "av0_1"), (psAV, "av1_0")]):
                c()
            v_chain(0, 0, (psAV, "av1_1"))()
            v_chain(1, 0, (psQ, "psq"))()
            vwork0 = [v_chain(st, 0) for st in range(2, ST)]
            vwork1 = [v_chain(st, 1) for st in range(ST)]

            # fill units per pair: pair j computes pair j+1's chains except
            # its k nn=1 chain, which runs early IN pair j+1 itself (the
            # nn=1 k-stationary is only needed from scores step 4) — this
            # keeps every pair boundary, including pair 7's, PE-dense;
            # v half-0 spread through pair 0 (needed by pair-0 AV), v
            # half-1 over pairs 1-3 (needed by pair 4)
            qku = {j: qk_alloc(j) for j in range(1, NPAIRS)}
            fill = {j: [] for j in range(NPAIRS)}
            fill[0] = vwork0 + qku[1]
            for j, nv in ((1, 2), (2, 2), (3, 2), (4, 2)):
                fill[j] = [vwork1.pop(0) for _ in range(nv)]
            for j in range(1, NPAIRS - 1):
                fill[j] = fill[j] + qku[j + 1]

            # spread sparse fill toward late/boundary steps (the PE dips
            # that re-throttle HAM happen at pair boundaries)
            POP_ORDER = [1, 3, 5, 7, 0, 2, 4, 6]
            for j in range(NPAIRS):
                work = list(fill[j])
                psteps = sorted(POP_ORDER[:min(len(work), ST)])
                for m in range(ST):
                    # AV of step m-2 is ready (its at-tile was exp'd two
                    # steps ago): emit it AHEAD of the exp-gated scores so
                    # the in-order TE queue never stalls behind a waiting
                    # scores matmul
                    if m == 0:
                        if j > 0:
                            rb_fin(j - 1, 1)
                        av_alloc(j)
                    if m >= 2:
                        av_m(j, m - 2)
                    # ready fill chains also go AHEAD of the exp-gated
                    # scores matmuls in the in-order TE queue
                    if work and (m in psteps or len(work) > ST - m):
                        work.pop(0)()
                    scores_exp(j, m)
                    if m == 6:
                        rb_prep(j, 0)
                    if m == 7:
                        rb_fin(j, 0)
                av_m(j, ST - 2)
                while work:
                    work.pop(0)()
                av_m(j, ST - 1)
                rb_prep(j, 1)
                if j == NPAIRS - 1:
                    # finish pair 7's normalization immediately, then
                    # prestart eight proj chunks (ko 0..6) on the freed
                    # psS/psQ/av banks while pair 7's tail drains
                    rb_fin(j, 1)
                    proj_mm(0, 0, range(7), psS, "ps")
                    proj_mm(0, 1, range(7), psS, "ps")
                    proj_mm(1, 0, range(7), psS, "ps")
                    proj_mm(1, 1, range(7), psQ, "psq")
                    proj_mm(2, 0, range(7), psAV, "av0_0")
                    proj_mm(2, 1, range(7), psAV, "av1_0")
                    proj_mm(3, 0, range(7), psAV, "av0_1")
                    proj_mm(3, 1, range(7), psAV, "av1_1")

            # ---------- output projection ----------
            slots = [(psQ, "psq"), (psAV, "av0_0"),
                     (psAV, "av0_1"), (psAV, "av1_0"), (psAV, "av1_1"),
                     (psS, "ps")]
            si = 0
            # finish the prestarted st=0..3 chunks, then st 4..7
            for st in range(ST):
                ystg = ystgp.tile([P, D], F32, tag="ystg", name=f"ystg{st}")
                for nE in range(2):
                    if st <= 3:
                        proj_mm(st, nE, range(7, 8), psS, "ps")
                    else:
                        pool, tag = slots[si % len(slots)]
                        proj_mm(st, nE, range(KO), pool, tag)
                        si += 1
                proj_evict(st, [ystg[:, 0:512], ystg[:, 512:1024]])
                nc.sync.dma_start(y_d[st * P:(st + 1) * P, :], ystg[:])

    nc.compile()
    return nc


def kernel(x, w_attn, b_attn, w_proj, b_proj):
    import concourse.bass_utils as bass_utils
    import ml_dtypes

    if "nc" not in _CACHE:
        _CACHE["nc"] = _build()
    nc = _CACHE["nc"]

    bf16 = ml_dtypes.bfloat16
    x = np.asarray(x, dtype=np.float32)
    w_attn = np.asarray(w_attn, dtype=np.float32)
    b_attn = np.asarray(b_attn, dtype=np.float32)
    w_proj = np.asarray(w_proj, dtype=np.float32)
    b_proj = np.asarray(b_proj, dtype=np.float32)

    xT = np.ascontiguousarray(
        np.transpose(x, (0, 2, 1))).astype(bf16)                 # [B, D, S]
    # interleave q/k per head pair: col block 256*j = [q m-tile j | k m-tile j]
    wqT = w_attn[:D].T.reshape(D, NPAIRS, P)                     # [D, 8, 128]
    wkT = w_attn[D:2 * D].T.reshape(D, NPAIRS, P)
    wqkT = np.ascontiguousarray(
        np.concatenate([wqT[:, :, None, :], wkT[:, :, None, :]], axis=2)
        .reshape(D, 2 * D)).astype(bf16)                         # [D, 2D]
    bq = b_attn[:D].reshape(NPAIRS, P)
    bk = b_attn[D:2 * D].reshape(NPAIRS, P)
    bqk = np.ascontiguousarray(
        np.concatenate([bq[:, None, :], bk[:, None, :]], axis=1).reshape(-1))
    wvT = np.ascontiguousarray(w_attn[2 * D:].T).astype(bf16)    # [D, D]
    wpT = np.ascontiguousarray(w_proj.T).astype(bf16)            # [D, D]
    bv = b_attn[2 * D:]
    beff = (b_proj.astype(np.float64)
            + w_proj.astype(np.float64) @ bv.astype(np.float64)
            ).astype(np.float32)
    beffb = np.ascontiguousarray(np.broadcast_to(beff, (P, D)))
    umask = np.triu(np.ones((P, P), dtype=np.float32)).astype(bf16)

    in_maps = [
        dict(xT=xT[b], wqkT=wqkT, wvT=wvT, wpT=wpT, bqk=bqk, beffb=beffb,
             umask=umask)
        for b in range(B)
    ]
    res = bass_utils.run_bass_kernel_spmd(
        nc, in_maps, core_ids=list(range(NCORES)), trace=TRACE)
    if TRACE:
        _CACHE["exec_time_ns"] = res.exec_time_ns
        _CACHE["trace"] = res.instructions_and_trace
    return np.stack([res.results[b]["y"] for b in range(B)], axis=0)
